# revision 2
# baseline (speedup 1.0000x reference)
"""Trainium2 Bass kernel for nn_ChannelPolyLayer.

out[b,o,x,y] = sum_c coeffs[b,o,c] * prod_v img[b,v,x,y] ** powers[c,v]
with degree<=3 trivariate monomials (20 coeffs), img channels (u,v,w).

Strategy
  - Data parallel over batch: 16 batches -> 8 cores x 2 batches.
  - Per core, the 2 batches are folded onto the partition axis:
    partitions 0..63 hold batch 0's pixel rows, 64..127 batch 1's.
    Per-partition coefficient APs then encode batch-dependent scalars,
    so a single SPMD program serves all cores and batches.
  - Exact factored evaluation (no pow):
        out_o = u*A_o(u,v,w) + v*D_o(v,w) + w*E_o(w) + c0_o
    A_o: inhomogeneous quadratic (10 coeffs), D_o: quadratic in (v,w) (6),
    E_o: quadratic in w (3), c0_o const.  Chains run as fused
    (mono*coeff)+acc scalar_tensor_tensor ops.
  - Work is split across engines: ScalarE does squares + chain heads,
    VectorE does the A-chains + products, GpSimd does crosses, D/E chains
    and the final sums, so the three engines run concurrently.
"""

import numpy as np

N_CORES = 8
BATCH, NVARS, H, W = 16, 3, 512, 512
NPIX = H * W            # 262144
P = 128
BPC = BATCH // N_CORES  # 2 batches per core
ROWS = P // BPC         # 64 partition rows per batch
COLS = NPIX // ROWS     # 4096 columns per plane
CW = 1024               # chunk width
NCHUNK = COLS // CW     # 4
NOUT = 3

# Coefficient-table layout per output o (20 columns each, 60 total):
# A (10): [const, u, v, w, u2, uv, uw, v2, vw, w2]
# D (6):  [const, v, w, v2, vw, w2]
# E (3):  [const, w, w2]
# c0 (1)
A_BASIS = [(0, 0, 0), (1, 0, 0), (0, 1, 0), (0, 0, 1), (2, 0, 0),
           (1, 1, 0), (1, 0, 1), (0, 2, 0), (0, 1, 1), (0, 0, 2)]
D_BASIS = [(0, 0, 0), (0, 1, 0), (0, 0, 1), (0, 2, 0), (0, 1, 1), (0, 0, 2)]
E_BASIS = [(0, 0, 0), (0, 0, 1), (0, 0, 2)]
TAB_COLS = NOUT * 20  # 60


def _coeff_table(coeffs_core: np.ndarray, powers: np.ndarray) -> np.ndarray:
    """coeffs_core [BPC, NOUT, 20] -> [P, TAB_COLS]; rows 0..63 batch0, 64.. batch1."""
    pw = [tuple(int(round(x)) for x in row) for row in np.asarray(powers)]
    tab = np.zeros((BPC, NOUT, 20), np.float64)
    a_idx = {m: i for i, m in enumerate(A_BASIS)}
    d_idx = {m: i for i, m in enumerate(D_BASIS)}
    e_idx = {m: i for i, m in enumerate(E_BASIS)}
    for b in range(BPC):
        for o in range(NOUT):
            for c, (pu, pv, pwz) in enumerate(pw):
                val = float(coeffs_core[b, o, c])
                if pu > 0:
                    tab[b, o, a_idx[(pu - 1, pv, pwz)]] += val
                elif pv > 0:
                    tab[b, o, 10 + d_idx[(0, pv - 1, pwz)]] += val
                elif pwz > 0:
                    tab[b, o, 16 + e_idx[(0, 0, pwz - 1)]] += val
                else:
                    tab[b, o, 19] += val
    out = np.empty((P, TAB_COLS), np.float32)
    for b in range(BPC):
        out[b * ROWS:(b + 1) * ROWS, :] = tab[b].reshape(1, TAB_COLS)
    return out


_NC_CACHE = {}


def _build_nc():
    if "nc" in _NC_CACHE:
        return _NC_CACHE["nc"]
    import concourse.mybir as mybir
    from concourse import bacc
    from concourse.tile import TileContext

    F32 = mybir.dt.float32
    MUL = mybir.AluOpType.mult
    ADD = mybir.AluOpType.add
    IDENT = mybir.ActivationFunctionType.Identity

    nc = bacc.Bacc("TRN2", target_bir_lowering=False)
    img = nc.dram_tensor("img", [NVARS, P, COLS], F32, kind="ExternalInput")
    ctab = nc.dram_tensor("ctab", [P, TAB_COLS], F32, kind="ExternalInput")
    out = nc.dram_tensor("out", [NOUT, P, COLS], F32, kind="ExternalOutput")

    with TileContext(nc) as tc:
        with (
            tc.tile_pool(name="tabp", bufs=1) as tabp,
            tc.tile_pool(name="inp", bufs=2) as inp,
            tc.tile_pool(name="sqp", bufs=2) as sqp,
            tc.tile_pool(name="crp", bufs=2) as crp,
            tc.tile_pool(name="chain", bufs=1) as chain,
            tc.tile_pool(name="prod", bufs=1) as prod,
            tc.tile_pool(name="outp", bufs=2) as outp,
        ):
            tab = tabp.tile([P, TAB_COLS], F32)
            nc.sync.dma_start(out=tab, in_=ctab[:, :])

            def col(o, k):
                j = o * 20 + k
                return tab[:, j:j + 1]

            for chk in range(NCHUNK):
                c0, c1 = chk * CW, (chk + 1) * CW
                u = inp.tile([P, CW], F32, tag="u")
                v = inp.tile([P, CW], F32, tag="v")
                w = inp.tile([P, CW], F32, tag="w")
                nc.sync.dma_start(out=u, in_=img[0, :, c0:c1])
                nc.sync.dma_start(out=v, in_=img[1, :, c0:c1])
                nc.sync.dma_start(out=w, in_=img[2, :, c0:c1])

                u2 = sqp.tile([P, CW], F32, tag="u2")
                v2 = sqp.tile([P, CW], F32, tag="v2")
                w2 = sqp.tile([P, CW], F32, tag="w2")
                nc.scalar.square(u2, u)
                nc.scalar.square(v2, v)
                nc.scalar.square(w2, w)
                uv = crp.tile([P, CW], F32, tag="uv")
                uw = crp.tile([P, CW], F32, tag="uw")
                vw = crp.tile([P, CW], F32, tag="vw")
                nc.gpsimd.tensor_mul(out=uv, in0=u, in1=v)
                nc.gpsimd.tensor_mul(out=uw, in0=u, in1=w)
                nc.gpsimd.tensor_mul(out=vw, in0=v, in1=w)

                a_mono = [v, w, u2, uv, uw, v2, vw, w2]
                d_mono = [w, v2, vw, w2]

                ats, dts, ets = {}, {}, {}
                for o in range(NOUT):
                    # A/D/E chains: heads on ACT, bodies fused stt on DVE
                    at = chain.tile([P, CW], F32, tag=f"at{o}", name=f"at{o}")
                    nc.scalar.activation(out=at, in_=u, func=IDENT,
                                         bias=col(o, 0), scale=col(o, 1))
                    for i, m in enumerate(a_mono):
                        nc.vector.scalar_tensor_tensor(
                            out=at, in0=m, scalar=col(o, 2 + i),
                            in1=at, op0=MUL, op1=ADD)
                    dt_ = chain.tile([P, CW], F32, tag=f"dt{o}", name=f"dt{o}")
                    nc.scalar.activation(out=dt_, in_=v, func=IDENT,
                                         bias=col(o, 10), scale=col(o, 11))
                    for i, m in enumerate(d_mono):
                        nc.vector.scalar_tensor_tensor(
                            out=dt_, in0=m, scalar=col(o, 12 + i),
                            in1=dt_, op0=MUL, op1=ADD)
                    et = chain.tile([P, CW], F32, tag=f"et{o}", name=f"et{o}")
                    nc.scalar.activation(out=et, in_=w, func=IDENT,
                                         bias=col(o, 16), scale=col(o, 17))
                    nc.vector.scalar_tensor_tensor(
                        out=et, in0=w2, scalar=col(o, 18),
                        in1=et, op0=MUL, op1=ADD)
                    ats[o], dts[o], ets[o] = at, dt_, et

                for o in range(NOUT):
                    # products and partial sum on GpSimd, final fused add on DVE
                    p1 = prod.tile([P, CW], F32, tag=f"p1{o}", name=f"p1{o}")
                    p2 = prod.tile([P, CW], F32, tag=f"p2{o}", name=f"p2{o}")
                    p3 = prod.tile([P, CW], F32, tag=f"p3{o}", name=f"p3{o}")
                    nc.gpsimd.tensor_mul(out=p1, in0=u, in1=ats[o])
                    nc.gpsimd.tensor_mul(out=p2, in0=v, in1=dts[o])
                    nc.gpsimd.tensor_mul(out=p3, in0=w, in1=ets[o])
                    s = prod.tile([P, CW], F32, tag=f"s{o}", name=f"s{o}")
                    nc.gpsimd.tensor_add(out=s, in0=p1, in1=p2)
                    ot = outp.tile([P, CW], F32, tag=f"ot{o}", name=f"ot{o}")
                    nc.vector.scalar_tensor_tensor(
                        out=ot, in0=p3, scalar=col(o, 19),
                        in1=s, op0=ADD, op1=ADD)
                    nc.sync.dma_start(out=out[o, :, c0:c1], in_=ot)
    nc.finalize()
    _NC_CACHE["nc"] = nc
    return nc


def _shard_core(img: np.ndarray, c: int) -> np.ndarray:
    """img [BATCH,3,H,W] -> per-core [NVARS, P, COLS] with batch on partitions."""
    blk = np.empty((NVARS, P, COLS), np.float32)
    for b in range(BPC):
        plane = img[c * BPC + b].reshape(NVARS, ROWS, COLS)
        blk[:, b * ROWS:(b + 1) * ROWS, :] = plane
    return blk


def kernel(img: np.ndarray, coeffs: np.ndarray, powers: np.ndarray) -> np.ndarray:
    from concourse.bass_utils import run_bass_kernel_spmd

    img = np.ascontiguousarray(np.asarray(img, np.float32))
    coeffs = np.asarray(coeffs, np.float32)
    powers = np.asarray(powers, np.float32)

    nc = _build_nc()
    in_maps = []
    for c in range(N_CORES):
        in_maps.append({
            "img": _shard_core(img, c),
            "ctab": _coeff_table(coeffs[c * BPC:(c + 1) * BPC], powers),
        })

    res = run_bass_kernel_spmd(nc, in_maps, core_ids=list(range(N_CORES)))
    _NC_CACHE["last_result"] = res
    out = np.empty((BATCH, NOUT, H, W), np.float32)
    for c in range(N_CORES):
        blk = res.results[c]["out"]  # [NOUT, P, COLS]
        for b in range(BPC):
            out[c * BPC + b] = blk[:, b * ROWS:(b + 1) * ROWS, :].reshape(NOUT, H, W)
    return out



# revision 3
# speedup vs baseline: 771.8235x; 771.8235x over previous
"""Trainium2 Bass kernel for nn_ChannelPolyLayer.

out[b,o,x,y] = sum_c coeffs[b,o,c] * prod_v img[b,v,x,y] ** powers[c,v]
with degree<=3 trivariate monomials (20 coeffs), img channels (u,v,w).

Strategy
  - Data parallel over batch: 16 batches -> 8 cores x 2 batches.
  - Per core, the 2 batches are folded onto the partition axis:
    partitions 0..63 hold batch 0's pixel rows, 64..127 batch 1's.
    Per-partition coefficient APs then encode batch-dependent scalars,
    so a single SPMD program serves all cores and batches.
  - Exact factored evaluation (no pow):
        out_o = u*A_o(u,v,w) + v*D_o(v,w) + w*E_o(w) + c0_o
    A_o: inhomogeneous quadratic (10 coeffs), D_o: quadratic in (v,w) (6),
    E_o: quadratic in w (3), c0_o const.  Chains run as fused
    (mono*coeff)+acc scalar_tensor_tensor ops.
  - Work is split across engines: ScalarE does squares + chain heads,
    VectorE does the A-chains + products, GpSimd does crosses, D/E chains
    and the final sums, so the three engines run concurrently.
"""

import numpy as np

N_CORES = 8
BATCH, NVARS, H, W = 16, 3, 512, 512
NPIX = H * W            # 262144
P = 128
BPC = BATCH // N_CORES  # 2 batches per core
ROWS = P // BPC         # 64 partition rows per batch
COLS = NPIX // ROWS     # 4096 columns per plane
CW = 1024               # chunk width
NCHUNK = COLS // CW     # 4
NOUT = 3

# Coefficient-table layout per output o (20 columns each, 60 total):
# A (10): [const, u, v, w, u2, uv, uw, v2, vw, w2]
# D (6):  [const, v, w, v2, vw, w2]
# E (3):  [const, w, w2]
# c0 (1)
A_BASIS = [(0, 0, 0), (1, 0, 0), (0, 1, 0), (0, 0, 1), (2, 0, 0),
           (1, 1, 0), (1, 0, 1), (0, 2, 0), (0, 1, 1), (0, 0, 2)]
D_BASIS = [(0, 0, 0), (0, 1, 0), (0, 0, 1), (0, 2, 0), (0, 1, 1), (0, 0, 2)]
E_BASIS = [(0, 0, 0), (0, 0, 1), (0, 0, 2)]
TAB_COLS = NOUT * 20  # 60


def _coeff_table(coeffs_core: np.ndarray, powers: np.ndarray) -> np.ndarray:
    """coeffs_core [BPC, NOUT, 20] -> [P, TAB_COLS]; rows 0..63 batch0, 64.. batch1."""
    pw = [tuple(int(round(x)) for x in row) for row in np.asarray(powers)]
    tab = np.zeros((BPC, NOUT, 20), np.float64)
    a_idx = {m: i for i, m in enumerate(A_BASIS)}
    d_idx = {m: i for i, m in enumerate(D_BASIS)}
    e_idx = {m: i for i, m in enumerate(E_BASIS)}
    for b in range(BPC):
        for o in range(NOUT):
            for c, (pu, pv, pwz) in enumerate(pw):
                val = float(coeffs_core[b, o, c])
                if pu > 0:
                    tab[b, o, a_idx[(pu - 1, pv, pwz)]] += val
                elif pv > 0:
                    tab[b, o, 10 + d_idx[(0, pv - 1, pwz)]] += val
                elif pwz > 0:
                    tab[b, o, 16 + e_idx[(0, 0, pwz - 1)]] += val
                else:
                    tab[b, o, 19] += val
    out = np.empty((P, TAB_COLS), np.float32)
    for b in range(BPC):
        out[b * ROWS:(b + 1) * ROWS, :] = tab[b].reshape(1, TAB_COLS)
    return out


_NC_CACHE = {}


def _build_nc():
    if "nc" in _NC_CACHE:
        return _NC_CACHE["nc"]
    import concourse.mybir as mybir
    from concourse import bacc
    from concourse.tile import TileContext

    F32 = mybir.dt.float32
    MUL = mybir.AluOpType.mult
    ADD = mybir.AluOpType.add
    IDENT = mybir.ActivationFunctionType.Identity

    nc = bacc.Bacc("TRN2", target_bir_lowering=False)
    img = nc.dram_tensor("img", [NVARS, P, COLS], F32, kind="ExternalInput")
    ctab = nc.dram_tensor("ctab", [P, TAB_COLS], F32, kind="ExternalInput")
    out = nc.dram_tensor("out", [NOUT, P, COLS], F32, kind="ExternalOutput")

    with TileContext(nc) as tc:
        with (
            tc.tile_pool(name="tabp", bufs=1) as tabp,
            tc.tile_pool(name="inp", bufs=2) as inp,
            tc.tile_pool(name="sqp", bufs=2) as sqp,
            tc.tile_pool(name="crp", bufs=2) as crp,
            tc.tile_pool(name="chain", bufs=1) as chain,
            tc.tile_pool(name="prod", bufs=1) as prod,
            tc.tile_pool(name="outp", bufs=2) as outp,
        ):
            tab = tabp.tile([P, TAB_COLS], F32)
            nc.sync.dma_start(out=tab, in_=ctab[:, :])

            def col(o, k):
                j = o * 20 + k
                return tab[:, j:j + 1]

            for chk in range(NCHUNK):
                c0, c1 = chk * CW, (chk + 1) * CW
                u = inp.tile([P, CW], F32, tag="u")
                v = inp.tile([P, CW], F32, tag="v")
                w = inp.tile([P, CW], F32, tag="w")
                nc.sync.dma_start(out=u, in_=img[0, :, c0:c1])
                nc.sync.dma_start(out=v, in_=img[1, :, c0:c1])
                nc.sync.dma_start(out=w, in_=img[2, :, c0:c1])

                u2 = sqp.tile([P, CW], F32, tag="u2")
                v2 = sqp.tile([P, CW], F32, tag="v2")
                w2 = sqp.tile([P, CW], F32, tag="w2")
                nc.scalar.square(u2, u)
                nc.scalar.square(v2, v)
                nc.scalar.square(w2, w)
                uv = crp.tile([P, CW], F32, tag="uv")
                uw = crp.tile([P, CW], F32, tag="uw")
                vw = crp.tile([P, CW], F32, tag="vw")
                nc.gpsimd.tensor_mul(out=uv, in0=u, in1=v)
                nc.gpsimd.tensor_mul(out=uw, in0=u, in1=w)
                nc.gpsimd.tensor_mul(out=vw, in0=v, in1=w)

                a_mono = [v, w, u2, uv, uw, v2, vw, w2]
                d_mono = [w, v2, vw, w2]

                ats, dts, ets = {}, {}, {}
                for o in range(NOUT):
                    # A/D/E chains: heads on ACT, bodies fused stt on DVE
                    at = chain.tile([P, CW], F32, tag=f"at{o}", name=f"at{o}")
                    nc.scalar.activation(out=at, in_=u, func=IDENT,
                                         bias=col(o, 0), scale=col(o, 1))
                    for i, m in enumerate(a_mono):
                        nc.vector.scalar_tensor_tensor(
                            out=at, in0=m, scalar=col(o, 2 + i),
                            in1=at, op0=MUL, op1=ADD)
                    dt_ = chain.tile([P, CW], F32, tag=f"dt{o}", name=f"dt{o}")
                    nc.scalar.activation(out=dt_, in_=v, func=IDENT,
                                         bias=col(o, 10), scale=col(o, 11))
                    for i, m in enumerate(d_mono):
                        nc.vector.scalar_tensor_tensor(
                            out=dt_, in0=m, scalar=col(o, 12 + i),
                            in1=dt_, op0=MUL, op1=ADD)
                    et = chain.tile([P, CW], F32, tag=f"et{o}", name=f"et{o}")
                    nc.scalar.activation(out=et, in_=w, func=IDENT,
                                         bias=col(o, 16), scale=col(o, 17))
                    nc.vector.scalar_tensor_tensor(
                        out=et, in0=w2, scalar=col(o, 18),
                        in1=et, op0=MUL, op1=ADD)
                    ats[o], dts[o], ets[o] = at, dt_, et

                for o in range(NOUT):
                    # products and partial sum on GpSimd, final fused add on DVE
                    p1 = prod.tile([P, CW], F32, tag=f"p1{o}", name=f"p1{o}")
                    p2 = prod.tile([P, CW], F32, tag=f"p2{o}", name=f"p2{o}")
                    p3 = prod.tile([P, CW], F32, tag=f"p3{o}", name=f"p3{o}")
                    nc.gpsimd.tensor_mul(out=p1, in0=u, in1=ats[o])
                    nc.gpsimd.tensor_mul(out=p2, in0=v, in1=dts[o])
                    nc.gpsimd.tensor_mul(out=p3, in0=w, in1=ets[o])
                    s = prod.tile([P, CW], F32, tag=f"s{o}", name=f"s{o}")
                    nc.gpsimd.tensor_add(out=s, in0=p1, in1=p2)
                    ot = outp.tile([P, CW], F32, tag=f"ot{o}", name=f"ot{o}")
                    nc.vector.scalar_tensor_tensor(
                        out=ot, in0=p3, scalar=col(o, 19),
                        in1=s, op0=ADD, op1=ADD)
                    nc.sync.dma_start(out=out[o, :, c0:c1], in_=ot)
    nc.finalize()
    _NC_CACHE["nc"] = nc
    return nc


def _shard_core(img: np.ndarray, c: int) -> np.ndarray:
    """img [BATCH,3,H,W] -> per-core [NVARS, P, COLS] with batch on partitions."""
    blk = np.empty((NVARS, P, COLS), np.float32)
    for b in range(BPC):
        plane = img[c * BPC + b].reshape(NVARS, ROWS, COLS)
        blk[:, b * ROWS:(b + 1) * ROWS, :] = plane
    return blk


def _make_in_maps(img, coeffs, powers):
    img = np.ascontiguousarray(np.asarray(img, np.float32))
    coeffs = np.asarray(coeffs, np.float32)
    powers = np.asarray(powers, np.float32)
    return [{
        "img": _shard_core(img, c),
        "ctab": _coeff_table(coeffs[c * BPC:(c + 1) * BPC], powers),
    } for c in range(N_CORES)]


def kernel(img: np.ndarray, coeffs: np.ndarray, powers: np.ndarray) -> np.ndarray:
    from concourse.bass_utils import run_bass_kernel_spmd

    nc = _build_nc()
    in_maps = _make_in_maps(img, coeffs, powers)

    res = run_bass_kernel_spmd(nc, in_maps, core_ids=list(range(N_CORES)))
    _NC_CACHE["last_result"] = res
    out = np.empty((BATCH, NOUT, H, W), np.float32)
    for c in range(N_CORES):
        blk = res.results[c]["out"]  # [NOUT, P, COLS]
        for b in range(BPC):
            out[c * BPC + b] = blk[:, b * ROWS:(b + 1) * ROWS, :].reshape(NOUT, H, W)
    return out



# revision 4
# speedup vs baseline: 11010.0634x; 14.2650x over previous
"""Trainium2 Bass kernel for nn_ChannelPolyLayer.

out[b,o,x,y] = sum_c coeffs[b,o,c] * prod_v img[b,v,x,y] ** powers[c,v]
with degree<=3 trivariate monomials (20 coeffs), img channels (u,v,w).

Strategy
  - Data parallel over batch: 16 batches -> 8 cores x 2 batches.
  - Per core, the 2 batches are folded onto the partition axis:
    partitions 0..63 hold batch 0's pixel rows, 64..127 batch 1's.
    Per-partition coefficient APs then encode batch-dependent scalars,
    so a single SPMD program serves all cores and batches.
  - Exact factored evaluation (no pow):
        out_o = u*A_o(u,v,w) + v*D_o(v,w) + w*E_o(w) + c0_o
    A_o: inhomogeneous quadratic (10 coeffs), D_o: quadratic in (v,w) (6),
    E_o: quadratic in w (3), c0_o const.  Chains run as fused
    (mono*coeff)+acc scalar_tensor_tensor ops.
  - Work is split across engines: ScalarE does squares + chain heads,
    VectorE does the A-chains + products, GpSimd does crosses, D/E chains
    and the final sums, so the three engines run concurrently.
"""

import os
import numpy as np

# Dev-only: repeat the compute body R times inside the NEFF so device time
# dominates the axon dispatch overhead during benchmarking. Unset => 1.
REPEAT = int(os.environ.get("POLY_BENCH_REPEAT", "1"))

N_CORES = 8
BATCH, NVARS, H, W = 16, 3, 512, 512
NPIX = H * W            # 262144
P = 128
BPC = BATCH // N_CORES  # 2 batches per core
ROWS = P // BPC         # 64 partition rows per batch
COLS = NPIX // ROWS     # 4096 columns per plane
CW = 1024               # chunk width
NCHUNK = COLS // CW     # 4
NOUT = 3

# Coefficient-table layout per output o (20 columns each, 60 total):
# A (10): [const, u, v, w, u2, uv, uw, v2, vw, w2]
# D (6):  [const, v, w, v2, vw, w2]
# E (3):  [const, w, w2]
# c0 (1)
A_BASIS = [(0, 0, 0), (1, 0, 0), (0, 1, 0), (0, 0, 1), (2, 0, 0),
           (1, 1, 0), (1, 0, 1), (0, 2, 0), (0, 1, 1), (0, 0, 2)]
D_BASIS = [(0, 0, 0), (0, 1, 0), (0, 0, 1), (0, 2, 0), (0, 1, 1), (0, 0, 2)]
E_BASIS = [(0, 0, 0), (0, 0, 1), (0, 0, 2)]
TAB_COLS = NOUT * 20  # 60


def _coeff_table(coeffs_core: np.ndarray, powers: np.ndarray) -> np.ndarray:
    """coeffs_core [BPC, NOUT, 20] -> [P, TAB_COLS]; rows 0..63 batch0, 64.. batch1."""
    pw = [tuple(int(round(x)) for x in row) for row in np.asarray(powers)]
    tab = np.zeros((BPC, NOUT, 20), np.float64)
    a_idx = {m: i for i, m in enumerate(A_BASIS)}
    d_idx = {m: i for i, m in enumerate(D_BASIS)}
    e_idx = {m: i for i, m in enumerate(E_BASIS)}
    for b in range(BPC):
        for o in range(NOUT):
            for c, (pu, pv, pwz) in enumerate(pw):
                val = float(coeffs_core[b, o, c])
                if pu > 0:
                    tab[b, o, a_idx[(pu - 1, pv, pwz)]] += val
                elif pv > 0:
                    tab[b, o, 10 + d_idx[(0, pv - 1, pwz)]] += val
                elif pwz > 0:
                    tab[b, o, 16 + e_idx[(0, 0, pwz - 1)]] += val
                else:
                    tab[b, o, 19] += val
    out = np.empty((P, TAB_COLS), np.float32)
    for b in range(BPC):
        out[b * ROWS:(b + 1) * ROWS, :] = tab[b].reshape(1, TAB_COLS)
    return out


_NC_CACHE = {}


def _build_nc():
    if "nc" in _NC_CACHE:
        return _NC_CACHE["nc"]
    import concourse.mybir as mybir
    from concourse import bacc
    from concourse.tile import TileContext

    F32 = mybir.dt.float32
    MUL = mybir.AluOpType.mult
    ADD = mybir.AluOpType.add
    IDENT = mybir.ActivationFunctionType.Identity

    nc = bacc.Bacc("TRN2", target_bir_lowering=False)
    img = nc.dram_tensor("img", [NVARS, P, COLS], F32, kind="ExternalInput")
    ctab = nc.dram_tensor("ctab", [P, TAB_COLS], F32, kind="ExternalInput")
    out = nc.dram_tensor("out", [NOUT, P, COLS], F32, kind="ExternalOutput")

    with TileContext(nc) as tc:
        with (
            tc.tile_pool(name="tabp", bufs=1) as tabp,
            tc.tile_pool(name="inp", bufs=2) as inp,
            tc.tile_pool(name="sqp", bufs=2) as sqp,
            tc.tile_pool(name="crp", bufs=2) as crp,
            tc.tile_pool(name="chain", bufs=1) as chain,
            tc.tile_pool(name="prod", bufs=1) as prod,
            tc.tile_pool(name="outp", bufs=2) as outp,
        ):
            tab = tabp.tile([P, TAB_COLS], F32)
            nc.sync.dma_start(out=tab, in_=ctab[:, :])

            def col(o, k):
                j = o * 20 + k
                return tab[:, j:j + 1]

            for chk in range(NCHUNK):
                c0, c1 = chk * CW, (chk + 1) * CW
                u = inp.tile([P, CW], F32, tag="u")
                v = inp.tile([P, CW], F32, tag="v")
                w = inp.tile([P, CW], F32, tag="w")
                nc.sync.dma_start(out=u, in_=img[0, :, c0:c1])
                nc.sync.dma_start(out=v, in_=img[1, :, c0:c1])
                nc.sync.dma_start(out=w, in_=img[2, :, c0:c1])

                u2 = sqp.tile([P, CW], F32, tag="u2")
                v2 = sqp.tile([P, CW], F32, tag="v2")
                w2 = sqp.tile([P, CW], F32, tag="w2")
                nc.scalar.square(u2, u)
                nc.scalar.square(v2, v)
                nc.scalar.square(w2, w)
                uv = crp.tile([P, CW], F32, tag="uv")
                uw = crp.tile([P, CW], F32, tag="uw")
                vw = crp.tile([P, CW], F32, tag="vw")
                nc.gpsimd.tensor_mul(out=uv, in0=u, in1=v)
                nc.gpsimd.tensor_mul(out=uw, in0=u, in1=w)
                nc.gpsimd.tensor_mul(out=vw, in0=v, in1=w)

                a_mono = [v, w, u2, uv, uw, v2, vw, w2]
                d_mono = [w, v2, vw, w2]

                ats, dts, ets = {}, {}, {}
                for o in range(NOUT):
                    # A/D/E chains: heads on ACT, bodies fused stt on DVE
                    at = chain.tile([P, CW], F32, tag=f"at{o}", name=f"at{o}")
                    nc.scalar.activation(out=at, in_=u, func=IDENT,
                                         bias=col(o, 0), scale=col(o, 1))
                    for i, m in enumerate(a_mono):
                        nc.vector.scalar_tensor_tensor(
                            out=at, in0=m, scalar=col(o, 2 + i),
                            in1=at, op0=MUL, op1=ADD)
                    dt_ = chain.tile([P, CW], F32, tag=f"dt{o}", name=f"dt{o}")
                    nc.scalar.activation(out=dt_, in_=v, func=IDENT,
                                         bias=col(o, 10), scale=col(o, 11))
                    for i, m in enumerate(d_mono):
                        nc.vector.scalar_tensor_tensor(
                            out=dt_, in0=m, scalar=col(o, 12 + i),
                            in1=dt_, op0=MUL, op1=ADD)
                    et = chain.tile([P, CW], F32, tag=f"et{o}", name=f"et{o}")
                    nc.scalar.activation(out=et, in_=w, func=IDENT,
                                         bias=col(o, 16), scale=col(o, 17))
                    nc.vector.scalar_tensor_tensor(
                        out=et, in0=w2, scalar=col(o, 18),
                        in1=et, op0=MUL, op1=ADD)
                    ats[o], dts[o], ets[o] = at, dt_, et

                for o in range(NOUT):
                    # products and partial sum on GpSimd, final fused add on DVE
                    p1 = prod.tile([P, CW], F32, tag=f"p1{o}", name=f"p1{o}")
                    p2 = prod.tile([P, CW], F32, tag=f"p2{o}", name=f"p2{o}")
                    p3 = prod.tile([P, CW], F32, tag=f"p3{o}", name=f"p3{o}")
                    nc.gpsimd.tensor_mul(out=p1, in0=u, in1=ats[o])
                    nc.gpsimd.tensor_mul(out=p2, in0=v, in1=dts[o])
                    nc.gpsimd.tensor_mul(out=p3, in0=w, in1=ets[o])
                    s = prod.tile([P, CW], F32, tag=f"s{o}", name=f"s{o}")
                    nc.gpsimd.tensor_add(out=s, in0=p1, in1=p2)
                    ot = outp.tile([P, CW], F32, tag=f"ot{o}", name=f"ot{o}")
                    nc.vector.scalar_tensor_tensor(
                        out=ot, in0=p3, scalar=col(o, 19),
                        in1=s, op0=ADD, op1=ADD)
                    nc.sync.dma_start(out=out[o, :, c0:c1], in_=ot)
    nc.finalize()
    _NC_CACHE["nc"] = nc
    return nc


def _shard_core(img: np.ndarray, c: int) -> np.ndarray:
    """img [BATCH,3,H,W] -> per-core [NVARS, P, COLS] with batch on partitions."""
    blk = np.empty((NVARS, P, COLS), np.float32)
    for b in range(BPC):
        plane = img[c * BPC + b].reshape(NVARS, ROWS, COLS)
        blk[:, b * ROWS:(b + 1) * ROWS, :] = plane
    return blk


def _make_in_maps(img, coeffs, powers):
    img = np.ascontiguousarray(np.asarray(img, np.float32))
    coeffs = np.asarray(coeffs, np.float32)
    powers = np.asarray(powers, np.float32)
    return [{
        "img": _shard_core(img, c),
        "ctab": _coeff_table(coeffs[c * BPC:(c + 1) * BPC], powers),
    } for c in range(N_CORES)]


def kernel(img: np.ndarray, coeffs: np.ndarray, powers: np.ndarray) -> np.ndarray:
    from concourse.bass_utils import run_bass_kernel_spmd

    nc = _build_nc()
    in_maps = _make_in_maps(img, coeffs, powers)

    res = run_bass_kernel_spmd(nc, in_maps, core_ids=list(range(N_CORES)))
    _NC_CACHE["last_result"] = res
    out = np.empty((BATCH, NOUT, H, W), np.float32)
    for c in range(N_CORES):
        blk = res.results[c]["out"]  # [NOUT, P, COLS]
        for b in range(BPC):
            out[c * BPC + b] = blk[:, b * ROWS:(b + 1) * ROWS, :].reshape(NOUT, H, W)
    return out

